# revision 46
# baseline (speedup 1.0000x reference)
"""Llama3 attention layer (T=2048, 32 q heads / 8 kv heads, D=128, hidden 4096)
on 8 Trainium2 NeuronCores, tensor-parallel over heads.

Per-core shard: 4 q heads + 1 kv head (w_qkv columns), 512 w_o rows.
Each core computes a full [T, 4096] o_proj partial in bf16; the host sums
the 8 partials in f32 (the all-reduce of the row-parallel w_o).

Device algorithm (per core), matmuls in bf16 with fp32 PSUM accumulation:
  1. qkv^T = w_shard^T @ hs^T          -> [768, T]  (c on partitions)
     tile 0 runs h-outer (6 PSUM banks, one per qkv column block) so the
     PE starts on the first ~320KB of DMA and streams; tiles 1-3 run
     cb-outer (k first, then q0..q3, then v)
  2. RoPE on q^T/k^T rows via duplicated cos/sin tables (DVE)
  3. V = transpose(v^T) via PE transposes (psum from the ps_o pool)
  4. S^T[s,t] = k^T.T @ q^T; two s-blocks share a 2-bank PSUM tile and a
     single exp on ACT (ACT has ~257ns fixed cost per instruction);
     0/1 mask on diagonal blocks (DVE)
  5. out^T[d,t] += V[s].T @ P^T[s,t]; denominator via DVE accumulation of
     P blocks + one ones-matmul per (head, tile); reciprocal via ACT
     ln/exp; normalize reads PSUM directly
  6. o_proj emitted as 32 "units" per tile, interleaved into the NEXT
     tile's QKV/attention phases to keep the PE saturated; the last tile
     holds a reserve of units to cover its final softmax chain

Schedule notes (from perfetto traces):
  - DMA issue rides TWO HWDGE queues (sync + scalar), each ring
    ~200GB/s: the initial w/hst stream is split across them in
    consumption order so the PE never outruns supply; cos/sin land
    mid-stream (a late rope table once stalled the whole DVE queue).
  - hst is double-buffered as half-tiles (3-buffer rotation) so the
    prefetch for tile j+1 starts mid-QKV(j) and lands long before use --
    avoids the PE idle + HAM re-warm ramp at tile boundaries.
  - the PE HAM throttles down when idle and re-warms at ~2x slower for
    ~8 matmuls; junk ident matmuls during the initial DMA wait start it
    warm, and fill pacing avoids mid-kernel idle.
  - the final t-block DMAs out per 512 (last unit per 256) columns so
    the tail after the last matmul is ~2us.
"""
import math
from collections import deque

import numpy as np
import ml_dtypes

import bass_rust
import concourse.bass as bass
import concourse.mybir as mybir
import concourse.tile as tile
from concourse.bass_utils import run_bass_kernel_spmd
from concourse.masks import make_identity
from concourse.vector_clock import ScopedClock

BF16 = mybir.dt.bfloat16
F32 = mybir.dt.float32
bf16 = ml_dtypes.bfloat16

T = 2048
HID = 4096
D = 128
NQH = 4           # q heads per core
CB = 6            # qkv col blocks of 128 (k, 4 q heads, v)
HCH = HID // 128  # 32 hidden chunks
TJ = 512          # t tile width
NJ = T // TJ      # 4 t tiles
SB = T // 128     # 16 s blocks
SCALE = 1.0 / math.sqrt(D)

_MAX_CTRL_WAITS = 1


def _install_drain_fix():
    """walrus in this image allows only 1 sem wait on CTRL (nop/drain)
    instructions; spread the Tile tail-drain's global-clock waits across
    preceding sync-engine NOPs."""
    if getattr(tile.TileContext, "_drain_fix_installed", False):
        return

    def _patched(self, tick_clock, wait_clock):
        nc = self.nc
        nops = [nc.sync.nop(nofuse=True, hint=f"drainw{i}") for i in range(32)]
        drain_inst = nc.sync.drain()
        wait_clock.add_sem_waits(
            drain_inst.ins, ScopedClock({None: tick_clock.global_clock})
        )
        si = drain_inst.ins.sync_info
        waits = list(si.on_wait) if si and si.on_wait else []
        if len(waits) > _MAX_CTRL_WAITS:
            chunks = [
                waits[i:i + _MAX_CTRL_WAITS]
                for i in range(0, len(waits), _MAX_CTRL_WAITS)
            ]
            drain_inst.ins.sync_info = bass_rust.SyncInfo(
                on_wait=chunks[-1], on_update=list(si.on_update or [])
            )
            for nop, chunk in zip(nops, chunks[:-1]):
                nop.ins.sync_info = bass_rust.SyncInfo(on_wait=chunk, on_update=[])
        nc.all_engine_barrier()
        assert self.sems is not None
        popped = nc._tile_sem_poison_stack.pop()
        assert popped is self._sem_poison
        nc.clear_and_free_semaphores(list(self.sems.allocated().values()))
        nc.all_engine_barrier()

    tile.TileContext._drain_and_barrier = _patched
    tile.TileContext._drain_fix_installed = True


def _fix_bir_waits(bir_json: bytes, max_waits: int = 1) -> bytes:
    """walrus in this image accepts very few sem waits per instruction.
    Split any instruction carrying more than `max_waits` waits by inserting
    same-engine NoOps ahead of it that carry the excess waits."""
    import json

    bir = json.loads(bir_json)
    for fn in bir["functions"]:
        for blk in fn["blocks"]:
            out = []
            for inst in blk["instructions"]:
                si = inst.get("sync_info")
                waits = (si or {}).get("on_wait") or []
                if len(waits) > max_waits:
                    chunks = [
                        waits[i:i + max_waits]
                        for i in range(0, len(waits), max_waits)
                    ]
                    for k, ch in enumerate(chunks[:-1]):
                        out.append(
                            {
                                "debug": inst.get("debug", 0),
                                "engine": inst["engine"],
                                "ins": [],
                                "name": f"{inst['name']}-w{k}",
                                "opcode": "NoOp",
                                "outs": [],
                                "sync_info": {"on_update": [], "on_wait": ch},
                            }
                        )
                    si["on_wait"] = chunks[-1]
                out.append(inst)
            blk["instructions"] = out
    return json.dumps(bir).encode()


def build_nc() -> bass.Bass:
    _install_drain_fix()
    nc = bass.Bass()

    # hsT[p, j, h, t]: hidden (h*128+p), t tile j col t  (contiguous per
    # (j, h-range) slice -> fast DMA).  w[p, h, cb, c]: h-major so one DMA
    # per h chunk grabs all 6 column blocks contiguously.
    hsT_d = nc.dram_tensor("hsT", [128, NJ, HCH, TJ], BF16, kind="ExternalInput")
    w_d = nc.dram_tensor("wqkv", [128, HCH, CB, 128], BF16, kind="ExternalInput")
    wo_d = nc.dram_tensor("wo", [128, NQH, HID], BF16, kind="ExternalInput")
    cos_d = nc.dram_tensor("cos2", [128, T], BF16, kind="ExternalInput")
    sin_d = nc.dram_tensor("sin2", [128, T], BF16, kind="ExternalInput")
    mask_d = nc.dram_tensor("masks", [128, TJ], BF16, kind="ExternalInput")
    out_d = nc.dram_tensor("out", [T, HID], BF16, kind="ExternalOutput")

    with tile.TileContext(nc) as tc:
        with (
            tc.tile_pool(name="const", bufs=1) as constp,
            tc.tile_pool(name="acts", bufs=1) as actp,
            tc.tile_pool(name="hst", bufs=3) as hstp,
            tc.tile_pool(name="qp", bufs=8) as qp,
            tc.tile_pool(name="otp", bufs=8) as otp,
            tc.tile_pool(name="Pp", bufs=2) as Pp,
            tc.tile_pool(name="qkt", bufs=2) as qktp,
            tc.tile_pool(name="rtmp", bufs=3) as rtp,
            tc.tile_pool(name="accp", bufs=4) as accp,
            tc.tile_pool(name="rcp", bufs=2) as rcpp,
            tc.tile_pool(name="vtp", bufs=2) as vtp,
            tc.tile_pool(name="outp", bufs=3) as outp,
            tc.tile_pool(name="pss", bufs=2, space="PSUM") as pssp,
            tc.tile_pool(name="pso", bufs=2, space="PSUM") as psop,
            tc.tile_pool(name="shp", bufs=2, space="PSUM") as shp,
        ):
            cos_sb = constp.tile([128, T], BF16, tag="cos")
            sin_sb = constp.tile([128, T], BF16, tag="sin")
            mask_sb = constp.tile([128, TJ], BF16, tag="mask")
            ones_sb = constp.tile([128, 128], BF16, tag="ones")
            ident_sb = constp.tile([128, 128], BF16, tag="ident")

            # persistent activations
            w_sb = actp.tile([128, HCH, CB, 128], BF16, tag="w")
            wo_sb = actp.tile([128, NQH, HID], BF16, tag="wo")
            kT_sb = actp.tile([128, T], BF16, tag="kT")
            v_sb = [
                actp.tile([128, 128], BF16, tag=f"v{i}", name=f"v{i}")
                for i in range(SB)
            ]

            # hst half-tiles: [128, 16, TJ] each, 3-buffer rotation.
            # halves[j][0] covers h 0..15, halves[j][1] covers h 16..31.
            hst_half = [[None, None] for _ in range(NJ)]

            def hst_rhs(j, h):
                half = hst_half[j][h // 16]
                return half[:, h % 16, :]

            def issue_hst(j, hh):
                t = hstp.tile([128, 16, TJ], BF16, tag="hst",
                              name=f"hst{j}_{hh}")
                hst_half[j][hh] = t
                nc.sync.dma_start(
                    t[:, 0:8, :], hsT_d[:, j, 16 * hh:16 * hh + 8, :]
                )
                nc.scalar.dma_start(
                    t[:, 8:16, :], hsT_d[:, j, 16 * hh + 8:16 * hh + 16, :]
                )

            # ---- initial DMA schedule: graduated consumption-ordered
            # stream so the first matmul starts on ~320KB of DMA and the
            # stream stays ahead of the PE.
            hst0a = hstp.tile([128, 16, TJ], BF16, tag="hst", name="hst0_0")
            hst0b = hstp.tile([128, 16, TJ], BF16, tag="hst", name="hst0_1")
            hst_half[0][0] = hst0a
            hst_half[0][1] = hst0b

            # two HWDGE queues (sync + scalar) in parallel, each ring
            # ~200GB/s: alternate (w_h, hst_h) chunk pairs across queues in
            # consumption order; rope tables / masks ride mid-stream; all
            # later prefetches split across both queues.
            def w_dma(h0, h1, eng):
                eng.dma_start(w_sb[:, h0:h1, :, :], w_d[:, h0:h1, :, :])

            def hst0_dma(h0, h1, eng):
                half, o = (hst0a, 0) if h0 < 16 else (hst0b, 16)
                eng.dma_start(
                    half[:, h0 - o:h1 - o, :], hsT_d[:, 0, h0:h1, :]
                )

            QA, QB = nc.sync, nc.scalar
            # QA (sync): w h0 + the full hst0 stream + masks + hst1_0 +
            # hst1_1[0:8].  QB (scalar): w h1..h31 + cos/sin + hst1_1[8:16].
            # ~8.1MB per queue at ~200GB/s/ring: everything attention(0)
            # needs lands by ~50us.
            w_dma(0, 1, QA)
            hst0_dma(0, 1, QA)
            w_dma(1, 2, QB)
            hst0_dma(1, 2, QA)
            w_dma(2, 4, QB)
            hst0_dma(2, 4, QA)
            w_dma(4, 8, QB)
            hst0_dma(4, 8, QA)
            w_dma(8, 12, QB)
            hst0_dma(8, 12, QA)
            QB.dma_start(cos_sb[:], cos_d[:])
            QB.dma_start(sin_sb[:], sin_d[:])
            w_dma(12, 16, QB)
            hst0_dma(12, 16, QA)
            w_dma(16, 20, QB)
            hst0_dma(16, 20, QA)
            w_dma(20, 24, QB)
            hst0_dma(20, 24, QA)
            w_dma(24, 28, QB)
            hst0_dma(24, 28, QA)
            w_dma(28, 32, QB)
            hst0_dma(28, 32, QA)
            QA.dma_start(mask_sb[:], mask_d[:])
            issue_hst(1, 0)
            nc.vector.memset(ones_sb[:], 1.0)
            make_identity(nc, ident_sb[:])

            # warm the PE (HAM throttles down when idle; re-warm costs ~8
            # half-speed matmuls): junk [128,128] matmuls during the
            # initial DMA wait so the first real matmul runs full speed
            ps_warm = shp.tile([128, 128], F32, tag="ps", name="ps_warm")
            for _ in range(24):
                nc.tensor.matmul(ps_warm[:], ident_sb[:], ident_sb[:],
                                 start=True, stop=True)

            # ---- o_proj unit machinery -------------------------------------
            # A unit computes out[t128, n*512:(n+1)*512] for one t block of
            # tile j from ot tiles + wo, staging into [128, HID/2] bf16
            # half-row tiles DMA'd per half (per 512-chunk for the final
            # t block so the last DMA lands right after the last matmul).
            pending = deque()
            ob_tiles = {}

            drain_mode = [False]
            drain_ctr = [0]

            def emit_unit():
                if not pending:
                    return False
                jj, tl, n, ot_tiles = pending.popleft()
                key = (jj, tl, n // 4)
                if key not in ob_tiles:
                    ob_tiles[key] = outp.tile(
                        [128, HID // 2], BF16, tag="ob",
                        name=f"ob{jj}_{tl}_{n // 4}"
                    )
                ob = ob_tiles[key]
                if drain_mode[0] and drain_ctr[0] % 2:
                    ps = pssp.tile([128, 2, TJ], F32, tag="ps",
                                   name="ps_u2")[:, 0, :]
                    psq = ps
                else:
                    psq = shp.tile([128, TJ], F32, tag="ps", name="ps_u")[:]
                    ps = psq
                drain_ctr[0] += 1
                tloc = bass.ts(tl, 128)
                for c in range(NQH):
                    nc.tensor.matmul(
                        ps, ot_tiles[c][:, tloc], wo_sb[:, c, bass.ts(n, TJ)],
                        start=(c == 0), stop=(c == NQH - 1),
                    )
                nn = n % 4
                tg = 4 * jj + tl
                last_block = (jj == NJ - 1 and tl == 3)
                if last_block:
                    # split the copy across ACT+DVE (both idle here) and
                    # DMA per 512-chunk: shortest last-matmul->done path.
                    # the very last unit DMAs per 256-col slice so the
                    # final DMA starts right after the second half's copy
                    nc.scalar.copy(ob[:, 512 * nn:512 * nn + 256],
                                   ps[:, 0:256])
                    if n == 7:
                        nc.sync.dma_start(
                            out_d[bass.ts(tg, 128), 512 * n:512 * n + 256],
                            ob[:, 512 * nn:512 * nn + 256],
                        )
                    nc.vector.tensor_copy(
                        ob[:, 512 * nn + 256:512 * nn + 512], ps[:, 256:512]
                    )
                    if n == 7:
                        nc.sync.dma_start(
                            out_d[bass.ts(tg, 128), 512 * n + 256:512 * n + 512],
                            ob[:, 512 * nn + 256:512 * nn + 512],
                        )
                    else:
                        nc.sync.dma_start(
                            out_d[bass.ts(tg, 128), bass.ts(n, TJ)],
                            ob[:, bass.ts(nn, TJ)],
                        )
                else:
                    if n % 2 == 0:
                        nc.scalar.copy(ob[:, bass.ts(nn, TJ)], ps)
                    else:
                        nc.vector.tensor_copy(ob[:, bass.ts(nn, TJ)], ps)
                    if nn == 3:
                        half = bass.ts(n // 4, HID // 2)
                        nc.sync.dma_start(out_d[bass.ts(tg, 128), half], ob[:])
                        del ob_tiles[key]
                return True

            # cb emission order: k first (rope for k completes while q
            # matmuls run), then q0..q3, then v.  Host packs w in this order.
            q_tiles_all = {}
            pending_finish = [None]

            def get_q_tiles(j):
                if j not in q_tiles_all:
                    q_tiles_all[j] = [
                        qp.tile([128, TJ], BF16, tag="q", name=f"q{j}_{h}")
                        for h in range(NQH)
                    ]
                return q_tiles_all[j]

            def run_finish():
                if pending_finish[0] is not None:
                    fin = pending_finish[0]
                    pending_finish[0] = None
                    fin()

            vt_tiles = {}

            def rope(ps, cb, j):
                """psum -> qkt copy -> rotary -> kT (cb==0) or q tile;
                cb==5 is v: plain copy to the per-tile vT staging tile."""
                js = bass.ts(j, TJ)
                if cb == 5:
                    vt = vtp.tile([128, TJ], BF16, tag="vt", name=f"vt{j}")
                    vt_tiles[j] = vt
                    nc.vector.tensor_copy(vt[:], ps[:])
                    return
                qk_t = qktp.tile([128, TJ], BF16, tag="qkt")
                nc.scalar.copy(qk_t[:], ps[:])
                swp = rtp.tile([128, TJ], BF16, tag="swp")
                nc.vector.tensor_copy(swp[0:64, :], qk_t[64:128, :])
                nc.vector.tensor_copy(swp[64:128, :], qk_t[0:64, :])
                ta = rtp.tile([128, TJ], BF16, tag="ta")
                nc.vector.tensor_tensor(
                    ta[:], qk_t[:], cos_sb[:, js], mybir.AluOpType.mult
                )
                tb = rtp.tile([128, TJ], BF16, tag="tb")
                nc.vector.tensor_tensor(
                    tb[:], swp[:], sin_sb[:, js], mybir.AluOpType.mult
                )
                dst = kT_sb[:, js] if cb == 0 else get_q_tiles(j)[cb - 1][:]
                nc.vector.tensor_tensor(dst, ta[:], tb[:], mybir.AluOpType.add)

            def emit_qkv_cb(j, cb, finish_at=None):
                ps = shp.tile([128, TJ], F32, tag="ps", name="ps_qkv")
                for h in range(HCH):
                    nc.tensor.matmul(
                        ps[:], w_sb[:, h, cb, :], hst_rhs(j, h),
                        start=(h == 0), stop=(h == HCH - 1),
                    )
                    if h == finish_at:
                        run_finish()
                rope(ps, cb, j)

            def emit_qkv_houter(j):
                """h-outer QKV for tile 0: 6 concurrent PSUM groups, one
                per column block, streaming the h chunks as they land.
                The last 4 h chunks run cb-outer so the ropes stagger."""
                pr0 = pssp.tile([128, 2, TJ], F32, tag="ps", name="ps0_p0")
                pr1 = pssp.tile([128, 2, TJ], F32, tag="ps", name="ps0_p1")
                pss = [pr0[:, 0, :], pr0[:, 1, :], pr1[:, 0, :], pr1[:, 1, :]] + [
                    shp.tile([128, TJ], F32, tag="ps", name=f"ps0_{c + 4}")[:]
                    for c in range(2)
                ]
                for h in range(HCH - 4):
                    for cb in range(CB):
                        nc.tensor.matmul(
                            pss[cb], w_sb[:, h, cb, :], hst_rhs(j, h),
                            start=(h == 0), stop=False,
                            skip_group_check=True,
                        )
                    if h == 15:
                        # half A fully consumed: start tile-1's second half
                        issue_hst(1, 1)
                for cb in (5, 0, 1, 2, 3, 4):
                    for h in range(HCH - 4, HCH):
                        nc.tensor.matmul(
                            pss[cb], w_sb[:, h, cb, :], hst_rhs(j, h),
                            start=False, stop=(h == HCH - 1),
                            skip_group_check=True,
                        )
                    rope(pss[cb], cb, j)

            def make_qkv_granules(j, cb):
                """Split one QKV column block into 4-matmul fill granules."""
                state = {}

                def mk(chunk):
                    def g():
                        if chunk == 0:
                            state["ps"] = shp.tile(
                                [128, TJ], F32, tag="ps", name="ps_qkv"
                            )
                        ps = state["ps"]
                        for h in range(4 * chunk, 4 * chunk + 4):
                            nc.tensor.matmul(
                                ps[:], w_sb[:, h, cb, :], hst_rhs(j, h),
                                start=(h == 0), stop=(h == HCH - 1),
                            )
                        if chunk == 7:
                            rope(state["ps"], cb, j)
                    return g

                return [mk(c) for c in range(8)]

            fill_q = deque()

            def fill(k):
                for _ in range(k):
                    if fill_q:
                        fill_q.popleft()()
                    elif not emit_unit():
                        return

            for j in range(NJ):
                js = bass.ts(j, TJ)
                nblk = 4 * j + 4
                ngrp = nblk // 2
                reserve = 8 if j == NJ - 1 else 0

                q_tiles = get_q_tiles(j)
                ot_tiles = [
                    otp.tile([128, TJ], BF16, tag="ot", name=f"ot{j}_{h}")
                    for h in range(NQH)
                ]

                # ---- QKV^T for this t tile (cb 0/1 of j=1 were pulled into
                # attention(0) as fill granules) ----
                if j == 0:
                    emit_qkv_houter(0)
                else:
                    first_cb = 2 if j == 1 else 0
                    for cb in range(first_cb, CB):
                        emit_qkv_cb(j, cb, finish_at=15 if cb == first_cb else None)

                # ---- V blocks for this tile ----
                vt = vt_tiles[j]
                for i in range(4 * j, 4 * j + 4):
                    pv = psop.tile([128, 128], BF16, tag="ps", name="ps_vt")
                    nc.tensor.transpose(
                        pv[:], vt[:, bass.ts(i - 4 * j, 128)], ident_sb[:]
                    )
                    nc.scalar.copy(v_sb[i][:], pv[:])

                # prefetch hst halves whose rotation buffer is now free:
                # after QKV(j), tile j's reads are done.  wo rides after the
                # hst(1) halves (first needed by attention(1)'s units).
                if j == 0:
                    nc.sync.dma_start(wo_sb[:, 0:2, :], wo_d[:, 0:2, :])
                    nc.scalar.dma_start(wo_sb[:, 2:4, :], wo_d[:, 2:4, :])
                    issue_hst(2, 0)
                elif j == 1:
                    issue_hst(2, 1)
                    issue_hst(3, 0)
                elif j == 2:
                    issue_hst(3, 1)
                if j == 0:
                    # att(0) has no o_proj units yet: fill it with the first
                    # two column blocks of QKV(1) instead
                    fill_q.extend(make_qkv_granules(1, 0))
                    fill_q.extend(make_qkv_granules(1, 1))

                # ---- attention: S-pairs one group ahead of PV; fill
                # (o_proj units / QKV granules) keeps the PE busy while
                # the ACT engine works through the exps; the last tile
                # holds `reserve` units back to cover the final finish ----
                slots_left = 4 * ngrp
                if j >= 1:
                    fill(4)
                for h in range(NQH):
                    qT = q_tiles[h]
                    P = Pp.tile([128, 6, TJ], BF16, tag="P", name=f"P{j}_{h}")
                    ps_o = psop.tile([128, TJ], F32, tag="ps", name="ps_o")
                    acc = [None, None]

                    def t0_of(i):
                        # causal: diagonal block i (s in [128i, 128i+128))
                        # only contributes to t >= 128(i-4j) within the tile
                        return 128 * (i - 4 * j) if i >= 4 * j else 0

                    def do_s_pair(p):
                        # two s blocks share a 2-bank psum tile and ONE exp
                        # (ACT has ~257ns fixed cost per instruction); the
                        # second block's [t00:t01) region holds exp(garbage)
                        # but is never read (PV/acc use each block's own t0)
                        i0, i1 = 2 * p, 2 * p + 1
                        t00, t01 = t0_of(i0), t0_of(i1)
                        ps2 = pssp.tile([128, 2, TJ], F32, tag="ps",
                                        name="ps_s2")
                        nc.tensor.matmul(
                            ps2[:, 0, t00:], kT_sb[:, bass.ts(i0, 128)],
                            qT[:, t00:], start=True, stop=True,
                        )
                        nc.tensor.matmul(
                            ps2[:, 1, t01:], kT_sb[:, bass.ts(i1, 128)],
                            qT[:, t01:], start=True, stop=True,
                        )
                        s0 = i0 % 6
                        nc.scalar.activation(
                            P[:, s0:s0 + 2, t00:], ps2[:, :, t00:],
                            mybir.ActivationFunctionType.Exp, scale=SCALE,
                        )
                        for i in (i0, i1):
                            t0 = t0_of(i)
                            if i >= 4 * j:
                                nc.vector.tensor_tensor(
                                    P[:, i % 6, t0:], P[:, i % 6, t0:],
                                    mask_sb[:, 0:TJ - t0],
                                    mybir.AluOpType.mult,
                                )

                    def do_pv(i):
                        t0 = t0_of(i)
                        nc.tensor.matmul(
                            ps_o[:, t0:], v_sb[i][:], P[:, i % 6, t0:],
                            start=(i == 0), stop=(i == nblk - 1),
                            skip_group_check=True,
                        )
                        a = i % 2
                        if acc[a] is None:
                            acc[a] = accp.tile(
                                [128, TJ], BF16, tag="acc", name=f"acc{a}"
                            )
                            if t0 == 0:
                                nc.vector.tensor_copy(acc[a][:], P[:, i % 6, :])
                            else:
                                nc.vector.memset(acc[a][:, 0:t0], 0.0)
                                nc.vector.tensor_copy(
                                    acc[a][:, t0:], P[:, i % 6, t0:]
                                )
                        else:
                            nc.vector.tensor_tensor(
                                acc[a][:, t0:], acc[a][:, t0:], P[:, i % 6, t0:],
                                mybir.AluOpType.add,
                            )

                    def do_slot():
                        resv = reserve if h < NQH - 1 else 4
                        items = max(
                            0, len(fill_q) + len(pending) - resv
                        )
                        k = -(-items // max(slots_left, 1))
                        fill(min(k, 3))

                    for p in range(nblk // 2):
                        do_s_pair(p)
                        if p == (3 if nblk >= 12 else 2):
                            run_finish()
                        if p >= 1:
                            do_slot()
                            slots_left -= 1
                            if p == 1:
                                fill(1)
                            do_pv(2 * p - 2)
                            do_pv(2 * p - 1)
                    do_slot()
                    slots_left -= 1
                    if nblk == 4:
                        run_finish()
                    do_pv(nblk - 2)
                    do_pv(nblk - 1)

                    def make_finish(h, acc, ps_o, ot_h):
                        def fin():
                            nc.vector.tensor_tensor(
                                acc[0][:], acc[0][:], acc[1][:],
                                mybir.AluOpType.add,
                            )
                            ps_den = shp.tile(
                                [128, TJ], F32, tag="ps", name="ps_den"
                            )
                            nc.tensor.matmul(
                                ps_den[:], ones_sb[:], acc[0][:],
                                start=True, stop=True,
                            )
                            # rc = 1/den via exp(-ln(den)) on ACT (ln and exp
                            # share a table; DVE InstReciprocal is 3.3us and
                            # custom-DVE/gpsimd ISA ops don't compile on this
                            # walrus)
                            ld = rcpp.tile([128, TJ], F32, tag="rc", name="ld")
                            nc.scalar.activation(
                                ld[:], ps_den[:],
                                mybir.ActivationFunctionType.Ln,
                            )
                            rc = rcpp.tile([128, TJ], F32, tag="rc", name="rc")
                            nc.scalar.activation(
                                rc[:], ld[:],
                                mybir.ActivationFunctionType.Exp, scale=-1.0,
                            )
                            nc.vector.tensor_tensor(
                                ot_h[:], ps_o[:], rc[:], mybir.AluOpType.mult
                            )
                        return fin

                    pending_finish[0] = make_finish(h, acc, ps_o, ot_tiles[h])

                # any unconsumed QKV granules must be emitted before the next
                # tile's S matmuls read the kT/q they produce
                while fill_q:
                    fill_q.popleft()()

                # queue this tile's o_proj units (consumed as fill in the
                # next tile's attention phase)
                for tl in range(4):
                    for n in range(HID // TJ):
                        pending.append((j, tl, n, ot_tiles))

            run_finish()
            drain_mode[0] = True
            while emit_unit():
                pass

    _orig_to_json = nc.to_json_bytes

    def _patched_to_json():
        return _fix_bir_waits(_orig_to_json())

    nc.to_json_bytes = _patched_to_json
    return nc


_NC_CACHE = None


def _get_nc():
    global _NC_CACHE
    if _NC_CACHE is None:
        _NC_CACHE = build_nc()
    return _NC_CACHE


def _host_prep(positions, hidden_states, w_qkv, w_o):
    H, HKV = 32, 8
    pos = np.asarray(positions).astype(np.float32)
    inv_freq = 1.0 / (500000.0 ** (np.arange(0, D, 2, dtype=np.float32) / D))
    freqs = pos[:, None] * inv_freq[None, :]                  # [T, 64]
    cos = np.cos(freqs).T                                     # [64, T]
    sin = np.sin(freqs).T
    cos2 = np.ascontiguousarray(
        np.concatenate([cos, cos], 0)
    ).astype(bf16)                                            # [128, T]
    sin2 = np.ascontiguousarray(np.concatenate([-sin, sin], 0)).astype(bf16)

    # causal 0/1 mask row: [p, f] = (p <= f); diagonal block r at column
    # offset 128r reads cols [0:512-128r] of this same row
    p = np.arange(128)[:, None]
    f = np.arange(TJ)[None, :]
    masks = np.ascontiguousarray(
        (p <= f).astype(np.float32)
    ).astype(bf16)                                            # [128, 512]

    hs = np.asarray(hidden_states)
    # [p, j, h, t]
    hsT = np.ascontiguousarray(
        hs.T.reshape(HCH, 128, NJ, TJ).transpose(1, 2, 0, 3)
    ).astype(bf16)
    w_qkv = np.asarray(w_qkv)
    w_o = np.asarray(w_o)

    in_maps = []
    for core in range(8):
        qc = slice(core * 4 * D, (core + 1) * 4 * D)
        kc = slice(H * D + core * D, H * D + (core + 1) * D)
        vc = slice((H + HKV) * D + core * D, (H + HKV) * D + (core + 1) * D)
        # cb order: k, q0..q3, v
        wshard = np.concatenate(
            [w_qkv[:, kc], w_qkv[:, qc], w_qkv[:, vc]], axis=1
        )                                                     # [4096, 768]
        # [p, h, cb, c]
        wshard = np.ascontiguousarray(
            wshard.reshape(HCH, 128, CB, 128).transpose(1, 0, 2, 3)
        ).astype(bf16)
        # [p, c, n]
        woshard = np.ascontiguousarray(
            w_o[core * 512:(core + 1) * 512, :]
            .reshape(NQH, 128, HID)
            .transpose(1, 0, 2)
        ).astype(bf16)
        in_maps.append(
            {
                "hsT": hsT,
                "wqkv": wshard,
                "wo": woshard,
                "cos2": cos2,
                "sin2": sin2,
                "masks": masks,
            }
        )
    return in_maps


def kernel(positions, hidden_states, w_qkv, w_o, _trace=False):
    nc = _get_nc()
    in_maps = _host_prep(positions, hidden_states, w_qkv, w_o)
    res = run_bass_kernel_spmd(nc, in_maps, list(range(8)), trace=_trace)
    out = np.zeros((T, HID), np.float32)
    for c in range(8):
        out += res.results[c]["out"].astype(np.float32)
    if _trace:
        kernel._last_result = res
    return out
